# revision 1
# baseline (speedup 1.0000x reference)
"""MixtureRouter Trainium2 kernel.

Per-core (data-parallel over batch, 8 cores): LayerNorm + Linear(2048->512)
+ GELU + sum-over-sequence + Linear(512->512) + router head, emitting one
row of router logits [1, 8]. Host gathers the 8 rows and computes the
scalar aux_loss and next_idx (trivial 64-element math).

Math notes (all exact rewrites of the reference):
  - gamma folds into w1 (w1g = gamma[:,None]*w1); beta@w1 + b1 folds into a
    free-dim vector vb1 added after the matmul.
  - LN mean-centering folds into the matmul as a rank-1 correction:
      (x - mu) @ w1g = x @ w1g - mu (x) u,   u = w1g.sum(0)
    applied as a K=1 matmul (outer product -mu (x) u) accumulated into the
    same PSUM bank.
  - the per-token rstd r scales the PSUM result; applied together with the
    +vb1 via one fused scalar_tensor_tensor op.
  - sum-over-tokens commutes with the second linear:
      sum_t(gelu_h @ w2 + b2) = (sum_t gelu_h) @ w2 + S*b2.
  - rsqrt computed on DVE with the bit-trick + 3 Newton steps (fp32-exact),
    avoiding ACT table-set thrash between Sqrt and Gelu.
"""

import sys
import types

import numpy as np

import concourse.bass as bass
import concourse.mybir as mybir
import concourse.tile as tile
from concourse import bacc
from concourse.bass_utils import run_bass_kernel_spmd
from concourse.masks import make_identity

# run_bass_kernel_spmd imports antenv.axon_hooks when BASS_TRACE is set; that
# module is absent on this image. Provide it so tracing degrades gracefully.
if "antenv.axon_hooks" not in sys.modules:
    try:
        import antenv.axon_hooks  # noqa: F401
    except ImportError:
        _hm = types.ModuleType("antenv.axon_hooks")
        _hm._hook = None
        _hm.set_axon_ntff_profile_hook = lambda h: setattr(_hm, "_hook", h)
        _hm.get_axon_ntff_profile_hook = lambda: _hm._hook
        sys.modules["antenv.axon_hooks"] = _hm
        try:
            from trn_agent_boot.trn_boot import _ntff_profile_via_ctypes

            _hm._hook = _ntff_profile_via_ctypes("/opt/axon/libaxon_pjrt.so")
        except Exception:
            pass

F32 = mybir.dt.float32
F32R = mybir.dt.float32r
I32 = mybir.dt.int32

B, S, D, R, E = 8, 2048, 2048, 512, 8
N_CORES = 8
P = 128
NT = S // P          # 16 token tiles per core
NK = D // P          # 16 contraction chunks
GRP = 4              # stat-processing group (tiles)
LN_EPS = 1e-5

_cache = {}


def _build():
    nc = bacc.Bacc("TRN2", target_bir_lowering=False, debug=False, num_devices=N_CORES)
    x = nc.dram_tensor("x", [S, D], F32, kind="ExternalInput")
    w1g = nc.dram_tensor("w1g", [D, R], F32R, kind="ExternalInput")
    u = nc.dram_tensor("u", [1, R], F32R, kind="ExternalInput")
    vb1 = nc.dram_tensor("vb1", [1, R], F32, kind="ExternalInput")
    w2 = nc.dram_tensor("w2", [R, R], F32, kind="ExternalInput")
    b2s = nc.dram_tensor("b2s", [R], F32, kind="ExternalInput")
    wr = nc.dram_tensor("wr", [R, E], F32, kind="ExternalInput")
    br = nc.dram_tensor("br", [1, E], F32, kind="ExternalInput")
    logits = nc.dram_tensor("logits", [1, E], F32, kind="ExternalOutput")

    with tile.TileContext(nc) as tc:
        with (
            tc.tile_pool(name="const", bufs=1) as const,
            tc.tile_pool(name="xin", bufs=3) as xin,
            tc.tile_pool(name="xtp", bufs=2) as xtp,
            tc.tile_pool(name="stats", bufs=2) as stats,
            tc.tile_pool(name="rows", bufs=1) as rows,
            tc.tile_pool(name="work", bufs=3) as work,
            tc.tile_pool(name="pst", bufs=2, space="PSUM") as pst,
            tc.tile_pool(name="psm", bufs=4, space="PSUM") as psm,
            tc.tile_pool(name="pssml", bufs=1, space="PSUM") as pssml,
        ):
            ident = const.tile([P, P], F32)
            make_identity(nc, ident)
            ones_col = const.tile([P, 1], F32)
            nc.vector.memset(ones_col, 1.0)

            w1g_s = const.tile([P, NK, R], F32R)
            nc.sync.dma_start(w1g_s, w1g.rearrange("(k p) n -> p k n", p=P))
            u_row = const.tile([1, R], F32R)
            nc.sync.dma_start(u_row, u[:, :])
            vb1b = const.tile([P, R], F32)
            nc.sync.dma_start(vb1b, vb1[0, :].partition_broadcast(P))
            w2_s = const.tile([P, 4, 4, P], F32)
            nc.sync.dma_start(w2_s, w2.rearrange("(rc p) (qc f) -> p rc qc f", p=P, f=P))
            b2s_s = const.tile([P, 4], F32)
            nc.sync.dma_start(b2s_s, b2s.rearrange("(c p) -> p c", p=P))
            wr_s = const.tile([P, 4, E], F32)
            nc.sync.dma_start(wr_s, wr.rearrange("(c p) e -> p c e", p=P))
            br_s = const.tile([1, E], F32)
            nc.sync.dma_start(br_s, br[:, :])

            gacc = const.tile([P, R], F32)
            nc.vector.memset(gacc, 0.0)

            # ---- fused per-tile loop: DMA -> PE transposes + matmul chain
            # (stats-independent) ; per group: stats -> corrections -> GELU
            mu_g = vr_g = None
            pm_tiles = [None] * NT
            for ti in range(NT):
                gi = ti % GRP
                g0 = ti - gi
                if gi == 0:
                    mu_g = stats.tile([P, GRP], F32, tag="mu")
                    vr_g = stats.tile([P, GRP], F32, tag="vr")
                xs = xin.tile([P, D], F32)
                nc.sync.dma_start(xs, x[ti * P:(ti + 1) * P, :])

                st = stats.tile([P, 4, 6], F32, tag="bn")
                for c in range(4):
                    nc.vector.bn_stats(st[:, c], xs[:, c * 512:(c + 1) * 512])
                mv = stats.tile([P, 2], F32, tag="mv")
                nc.vector.bn_aggr(mv, st)
                nc.vector.tensor_copy(mu_g[:, gi:gi + 1], mv[:, 0:1])
                nc.vector.tensor_copy(vr_g[:, gi:gi + 1], mv[:, 1:2])

                xT = xtp.tile([P, NK, P], F32R, tag="xT")
                for kg in range(4):
                    pt = pst.tile([P, 4, P], F32, tag="tp")
                    for j in range(4):
                        k = kg * 4 + j
                        nc.tensor.transpose(pt[:, j], xs[:, k * P:(k + 1) * P], ident)
                    if kg % 2 == 0:
                        nc.vector.tensor_copy(xT[:, kg * 4:(kg + 1) * 4], pt)
                    else:
                        nc.scalar.copy(xT[:, kg * 4:(kg + 1) * 4], pt)

                pm = psm.tile([P, R], F32, tag="mm")
                for k in range(NK):
                    nc.tensor.matmul(
                        pm, xT[:, k], w1g_s[:, k],
                        start=(k == 0), stop=False, skip_group_check=True,
                    )
                pm_tiles[ti] = pm

                if gi == GRP - 1:
                    # v = var + eps ; r = rsqrt(v) via bit trick + 2 Newton steps
                    v = stats.tile([P, GRP], F32, tag="v")
                    nc.vector.tensor_scalar_add(v, vr_g, LN_EPS)
                    r = stats.tile([P, GRP], F32, tag="r")
                    nc.vector.tensor_scalar(
                        r.bitcast(I32), v.bitcast(I32), 1, None,
                        op0=mybir.AluOpType.arith_shift_right,
                    )
                    nc.vector.tensor_scalar(
                        r.bitcast(I32), r.bitcast(I32), 0x5F3759DF, -1,
                        op0=mybir.AluOpType.subtract, op1=mybir.AluOpType.mult,
                    )
                    t = stats.tile([P, GRP], F32, tag="t")
                    for _ in range(2):
                        nc.vector.tensor_mul(t, r, r)
                        nc.vector.tensor_mul(t, t, v)
                        nc.vector.tensor_scalar(
                            t, t, -0.5, 1.5,
                            op0=mybir.AluOpType.mult, op1=mybir.AluOpType.add,
                        )
                        nc.vector.tensor_mul(r, r, t)
                    rg = stats.tile([P, GRP], F32, tag="rg")
                    nc.vector.tensor_copy(rg, r)
                    # negmu rows for the rank-1 LN correction: transpose each
                    # [128,1] stat column into a partition-0 [1,128] row
                    nm = stats.tile([P, GRP], F32, tag="nm")
                    nc.vector.tensor_scalar_mul(nm, mu_g, -1.0)
                    nmT_ps = pssml.tile([1, GRP, P], F32, tag="strow")
                    for j in range(GRP):
                        nc.tensor.transpose(nmT_ps[:, j], nm[:, j:j + 1], ident)
                    nmrow_g = rows.tile([1, GRP, P], F32R, tag=f"nmrow{g0 // GRP}")
                    nc.vector.tensor_copy(nmrow_g, nmT_ps)

                    for j in range(GRP):
                        tj = g0 + j
                        pmj = pm_tiles[tj]
                        nc.tensor.matmul(
                            pmj, nmrow_g[:, j, :], u_row,
                            start=False, stop=True, skip_group_check=True,
                        )
                        pre = work.tile([P, R], F32, tag="pre")
                        nc.vector.scalar_tensor_tensor(
                            pre, pmj, rg[:, j:j + 1], vb1b,
                            op0=mybir.AluOpType.mult, op1=mybir.AluOpType.add,
                        )
                        gt = work.tile([P, R], F32, tag="g")
                        nc.scalar.activation(
                            gt, pre, mybir.ActivationFunctionType.Gelu)
                        nc.vector.tensor_add(gacc, gacc, gt)

            # ---- tail: H = colsum(gacc); bt = H@w2 + S*b2; logits = bt@wr + br
            h_ps = pssml.tile([P, 4], F32, tag="tail")
            for rc in range(4):
                nc.tensor.matmul(
                    h_ps[:, rc:rc + 1], gacc[:, rc * P:(rc + 1) * P], ones_col,
                    start=True, stop=True,
                )
            h_s = rows.tile([P, 4], F32, tag="hs")
            nc.vector.tensor_copy(h_s, h_ps)

            bt_s = rows.tile([P, 4], F32, tag="bt")
            for qc in range(4):
                bt_ps = pssml.tile([P, 1], F32, tag="tail")
                for rc in range(4):
                    nc.tensor.matmul(
                        bt_ps, w2_s[:, rc, qc, :], h_s[:, rc:rc + 1],
                        start=(rc == 0), stop=(rc == 3),
                    )
                nc.vector.tensor_scalar_add(bt_s[:, qc:qc + 1], bt_ps, b2s_s[:, qc:qc + 1])

            lg_ps = pssml.tile([1, E], F32, tag="tail")
            for qc in range(4):
                nc.tensor.matmul(
                    lg_ps, bt_s[:, qc:qc + 1], wr_s[:, qc, :],
                    start=(qc == 0), stop=(qc == 3),
                )
            lg_s = rows.tile([1, E], F32, tag="lgs")
            nc.vector.tensor_add(lg_s, lg_ps, br_s)
            nc.sync.dma_start(logits[:, :], lg_s)
    nc.finalize()
    return nc


def kernel(hidden_states, ln_gamma, ln_beta, w1, b1, w2, b2, wr, br):
    hs = np.asarray(hidden_states, dtype=np.float32)
    g64 = np.asarray(ln_gamma, dtype=np.float64)
    be64 = np.asarray(ln_beta, dtype=np.float64)
    w1_64 = np.asarray(w1, dtype=np.float64)
    w1g = (g64[:, None] * w1_64).astype(np.float32)
    u = w1g.astype(np.float64).sum(0).astype(np.float32).reshape(1, R)
    vb1 = (be64 @ w1_64 + np.asarray(b1, np.float64)).astype(np.float32).reshape(1, R)
    b2s = (np.asarray(b2, np.float64) * float(S)).astype(np.float32)
    w2f = np.asarray(w2, dtype=np.float32)
    wrf = np.asarray(wr, dtype=np.float32)
    brf = np.asarray(br, dtype=np.float32).reshape(1, E)

    if "nc" not in _cache:
        _cache["nc"] = _build()
    nc = _cache["nc"]

    in_maps = []
    for b in range(N_CORES):
        in_maps.append({
            "x": np.ascontiguousarray(hs[b]),
            "w1g": w1g, "u": u, "vb1": vb1,
            "w2": w2f, "b2s": b2s, "wr": wrf, "br": brf,
        })
    res = run_bass_kernel_spmd(nc, in_maps, core_ids=list(range(N_CORES)))
    logits = np.concatenate([res.results[b]["logits"] for b in range(N_CORES)], axis=0)
    global _last_res, _last_logits
    _last_res = res
    _last_logits = logits

    l64 = logits.astype(np.float64)
    idx = l64.argmax(axis=-1)
    targets = np.zeros_like(l64)
    targets[np.arange(B), idx] = 1.0
    aux = (np.logaddexp(0.0, l64) - l64 * targets).mean()
    counts = targets.sum(0)
    next_idx = int(np.argmax(counts))
    return np.float32(aux), np.int32(next_idx)



# revision 8
# speedup vs baseline: 1.5824x; 1.5824x over previous
"""MixtureRouter Trainium2 kernel (v2).

Per-core (data-parallel over batch, 8 cores): LayerNorm + Linear(2048->512)
+ GELU + sum-over-sequence, emitting the per-core GELU accumulator
[128, 512] (token-slot x feature). Host sums the 128 token-slot rows,
applies the tiny tail (H @ w2 + S*b2 -> router head) and computes
aux_loss / next_idx in fp64 (trivial 512x512 math, exact rewrite).

Math notes (exact rewrites of the reference):
  - gamma folds into w1 (w1g = gamma[:,None]*w1); u = w1g.sum(0);
    vb1 = beta@w1 + b1.
  - LN folds into the matmul output: with pm0 = x @ w1g,
      pre = r*(x_c @ w1g) + vb1           (x_c = x - mu, r = rsqrt(var+eps))
          = r * (pm0 - mu (x) u + sqrt(var+eps) (x) vb1)
    so both correction terms form ONE K=2 rank-2 matmul accumulated into
    the same PSUM bank (stationary rows [-mu_t, sqrt(v)_t], moving rows
    [u, vb1]), and the r-scale rides the ACT Gelu as a per-partition
    `scale` operand. No elementwise correction passes at all.
  - sum-over-tokens commutes with the second linear:
      sum_t(gelu_h @ w2 + b2) = (sum_t gelu_h) @ w2 + S*b2  (host).
  - rsqrt via DVE bit-trick + 2 Newton steps (avoids ACT table thrash).

Schedule notes:
  - PE does the 256 transposes (LDW-bound ~110ns), 256 f32r matmuls
    (~225ns, 1cyc/row at free=512), 16 stat-row transposes and 16 K=2
    rank-2 matmuls. Software-pipelined one tile ahead: transposes of tile
    i+1 are issued before matmuls of tile i so the PSUM->SBUF copies (on
    ACT) never stall the PE.
  - DMA order: x tile 0 first (split in halves), then w1g in chunks, so
    the PE starts ~6us in instead of ~29us.
"""

import sys
import types

import numpy as np

import concourse.bass as bass
import concourse.mybir as mybir
import concourse.tile as tile
from concourse import bacc
from concourse.bass_utils import run_bass_kernel_spmd
from concourse.masks import make_identity

# run_bass_kernel_spmd imports antenv.axon_hooks when BASS_TRACE is set; that
# module is absent on this image. Provide it so tracing degrades gracefully.
if "antenv.axon_hooks" not in sys.modules:
    try:
        import antenv.axon_hooks  # noqa: F401
    except ImportError:
        _hm = types.ModuleType("antenv.axon_hooks")
        _hm._hook = None
        _hm.set_axon_ntff_profile_hook = lambda h: setattr(_hm, "_hook", h)
        _hm.get_axon_ntff_profile_hook = lambda: _hm._hook
        sys.modules["antenv.axon_hooks"] = _hm
        try:
            from trn_agent_boot.trn_boot import _ntff_profile_via_ctypes

            _hm._hook = _ntff_profile_via_ctypes("/opt/axon/libaxon_pjrt.so")
        except Exception:
            pass

F32 = mybir.dt.float32
F32R = mybir.dt.float32r
I32 = mybir.dt.int32

B, S, D, R, E = 8, 2048, 2048, 512, 8
N_CORES = 8
P = 128
NT = S // P          # 16 token tiles per core
NK = D // P          # 16 contraction chunks
GRP = 4              # stat-processing group (tiles)
LN_EPS = 1e-5

_cache = {}


def _build():
    nc = bacc.Bacc("TRN2", target_bir_lowering=False, debug=False, num_devices=N_CORES)
    x = nc.dram_tensor("x", [S, D], F32, kind="ExternalInput")
    w1g = nc.dram_tensor("w1g", [D, R], F32R, kind="ExternalInput")
    uvb = nc.dram_tensor("uvb", [2, R], F32R, kind="ExternalInput")
    gout = nc.dram_tensor("gout", [P, R], F32, kind="ExternalOutput")

    with tile.TileContext(nc) as tc:
        with (
            tc.tile_pool(name="const", bufs=1) as const,
            tc.tile_pool(name="xin", bufs=3) as xin,
            tc.tile_pool(name="xtp", bufs=2) as xtp,
            tc.tile_pool(name="stats", bufs=2) as stats,
            tc.tile_pool(name="rows", bufs=2) as rows,
            tc.tile_pool(name="work", bufs=3) as work,
            tc.tile_pool(name="pst", bufs=2, space="PSUM") as pst,
            tc.tile_pool(name="psm", bufs=4, space="PSUM") as psm,
            tc.tile_pool(name="pssml", bufs=2, space="PSUM") as pssml,
        ):
            # ---- startup-critical DMA ordering: x tile 0 halves first.
            xs_tiles = [None] * NT
            xs0 = xin.tile([P, NK, P], F32, tag="xs")
            nc.sync.dma_start(
                xs0[:, : NK // 2], x[0:P, : D // 2].rearrange("p (k q) -> p k q", q=P)
            )
            nc.sync.dma_start(
                xs0[:, NK // 2 :], x[0:P, D // 2 :].rearrange("p (k q) -> p k q", q=P)
            )
            xs_tiles[0] = xs0

            ident = const.tile([P, P], F32)
            make_identity(nc, ident)

            # w1g in 4 chunk-groups so k=0..3 arrive before the first matmuls
            w1g_s = const.tile([P, NK, R], F32R)
            for kg in range(4):
                nc.sync.dma_start(
                    w1g_s[:, kg * 4 : (kg + 1) * 4],
                    w1g[kg * 4 * P : (kg + 1) * 4 * P, :].rearrange(
                        "(k p) n -> p k n", p=P
                    ),
                )

            xs1 = xin.tile([P, NK, P], F32, tag="xs")
            nc.sync.dma_start(xs1, x[P : 2 * P, :].rearrange("p (k q) -> p k q", q=P))
            xs_tiles[1] = xs1

            uvb_s = const.tile([2, R], F32R)
            nc.sync.dma_start(uvb_s, uvb[:, :])

            gacc = const.tile([P, R], F32)
            nc.vector.memset(gacc, 0.0)

            # ---- per-tile emission, software-pipelined one tile ahead on PE
            mu_g = vr_g = None
            pm_tiles = [None] * NT
            xT_tiles = [None] * NT
            group_ready = [None] * (NT // GRP)   # (rg, row2) per group
            pending = []                         # tiles awaiting K2+postproc

            def emit_front(ti):
                """DMA (ti+2), stats(ti), transposes(ti) + copies(ti)."""
                if ti + 2 < NT:
                    xs_n = xin.tile([P, NK, P], F32, tag="xs")
                    nc.sync.dma_start(
                        xs_n,
                        x[(ti + 2) * P : (ti + 3) * P, :].rearrange(
                            "p (k q) -> p k q", q=P
                        ),
                    )
                    xs_tiles[ti + 2] = xs_n
                xs = xs_tiles[ti]

                st = stats.tile([P, 4, 6], F32, tag="bn")
                xs4 = xs.rearrange("p (a k) q -> p a (k q)", a=4)
                for c in range(4):
                    nc.vector.bn_stats(st[:, c], xs4[:, c])
                mv = stats.tile([P, 2], F32, tag="mv")
                nc.vector.bn_aggr(mv, st)
                gi = ti % GRP
                nc.vector.tensor_copy(mu_g[:, gi : gi + 1], mv[:, 0:1])
                nc.vector.tensor_copy(vr_g[:, gi : gi + 1], mv[:, 1:2])

                xT = xtp.tile([P, NK, P], F32R, tag="xT")
                for kg in range(4):
                    pt = pst.tile([P, 4, P], F32, tag="tp")
                    for j in range(4):
                        k = kg * 4 + j
                        nc.tensor.transpose(pt[:, j], xs[:, k], ident)
                    nc.scalar.copy(xT[:, kg * 4 : (kg + 1) * 4], pt)
                xT_tiles[ti] = xT

            def emit_matmuls(ti):
                pm = psm.tile([P, R], F32, tag="mm")
                xT = xT_tiles[ti]
                for k in range(NK):
                    nc.tensor.matmul(
                        pm, xT[:, k], w1g_s[:, k],
                        start=(k == 0), stop=False, skip_group_check=True,
                    )
                pm_tiles[ti] = pm

            def emit_group_math():
                """rsqrt + stat-row transposes for the current group."""
                v = stats.tile([P, GRP], F32, tag="v")
                nc.vector.tensor_scalar_add(v, vr_g, LN_EPS)
                r = stats.tile([P, GRP], F32, tag="r")
                nc.vector.tensor_scalar(
                    r.bitcast(I32), v.bitcast(I32), 1, None,
                    op0=mybir.AluOpType.arith_shift_right,
                )
                nc.vector.tensor_scalar(
                    r.bitcast(I32), r.bitcast(I32), 0x5F3759DF, -1,
                    op0=mybir.AluOpType.subtract, op1=mybir.AluOpType.mult,
                )
                t = stats.tile([P, GRP], F32, tag="t")
                for _ in range(2):
                    nc.vector.tensor_mul(t, r, r)
                    nc.vector.tensor_mul(t, t, v)
                    nc.vector.tensor_scalar(
                        t, t, -0.5, 1.5,
                        op0=mybir.AluOpType.mult, op1=mybir.AluOpType.add,
                    )
                    nc.vector.tensor_mul(r, r, t)
                rg = stats.tile([P, GRP], F32, tag="rg")
                nc.vector.tensor_copy(rg, r)
                # stationary rows for the K=2 correction: [-mu_j, sqrt(v)_j]
                nmsq = stats.tile([P, 2, GRP], F32, tag="nmsq")
                nc.vector.tensor_scalar_mul(nmsq[:, 0], mu_g, -1.0)
                nc.vector.tensor_mul(nmsq[:, 1], v, rg)
                strow = pssml.tile([2, GRP, P], F32, tag="strow")
                for j in range(GRP):
                    nc.tensor.transpose(strow[:, j], nmsq[:, :, j], ident)
                row2 = rows.tile([2, GRP, P], F32R, tag="row2")
                nc.vector.tensor_copy(row2, strow)
                return rg, row2

            def emit_postproc(ti):
                """K2 rank-2 correction (PE), Gelu*r (ACT), accumulate (DVE)."""
                gi = ti % GRP
                rg, row2 = group_ready[ti // GRP]
                nc.tensor.matmul(
                    pm_tiles[ti], row2[:, gi], uvb_s,
                    start=False, stop=True, skip_group_check=True,
                )
                gt = work.tile([P, R], F32, tag="g")
                nc.scalar.activation(
                    gt, pm_tiles[ti], mybir.ActivationFunctionType.Gelu,
                    scale=rg[:, gi : gi + 1],
                )
                pm_tiles[ti] = None
                nc.vector.tensor_add(gacc, gacc, gt)

            def flush_postproc():
                while pending and group_ready[pending[0] // GRP] is not None:
                    emit_postproc(pending.pop(0))

            for ti in range(NT):
                if ti % GRP == 0:
                    mu_g = stats.tile([P, GRP], F32, tag="mu")
                    vr_g = stats.tile([P, GRP], F32, tag="vr")
                emit_front(ti)
                if ti % GRP == GRP - 1:
                    group_ready[ti // GRP] = emit_group_math()
                if ti > 0:
                    emit_matmuls(ti - 1)
                    pending.append(ti - 1)
                    flush_postproc()
            emit_matmuls(NT - 1)
            pending.append(NT - 1)
            flush_postproc()
            assert not pending

            nc.sync.dma_start(gout[:, :], gacc)
    nc.finalize()
    return nc


def kernel(hidden_states, ln_gamma, ln_beta, w1, b1, w2, b2, wr, br):
    hs = np.asarray(hidden_states, dtype=np.float32)
    g64 = np.asarray(ln_gamma, dtype=np.float64)
    be64 = np.asarray(ln_beta, dtype=np.float64)
    w1_64 = np.asarray(w1, dtype=np.float64)
    w1g = (g64[:, None] * w1_64).astype(np.float32)
    u = w1g.astype(np.float64).sum(0).astype(np.float32)
    vb1 = (be64 @ w1_64 + np.asarray(b1, np.float64)).astype(np.float32)
    uvb = np.stack([u, vb1], axis=0)  # [2, R]

    if "nc" not in _cache:
        _cache["nc"] = _build()
    nc = _cache["nc"]

    in_maps = []
    for b in range(N_CORES):
        in_maps.append({
            "x": np.ascontiguousarray(hs[b]),
            "w1g": w1g, "uvb": uvb,
        })
    res = run_bass_kernel_spmd(nc, in_maps, core_ids=list(range(N_CORES)))
    gaccs = np.stack([res.results[b]["gout"] for b in range(N_CORES)], axis=0)
    global _last_res
    _last_res = res

    # host tail in fp64 (tiny): H -> w2 -> router -> aux/next_idx
    H = gaccs.astype(np.float64).sum(axis=1)                      # [B, R]
    bt = H @ np.asarray(w2, np.float64) + float(S) * np.asarray(b2, np.float64)
    logits = bt @ np.asarray(wr, np.float64) + np.asarray(br, np.float64)  # [B, E]
    global _last_logits
    _last_logits = logits.astype(np.float32)

    idx = logits.argmax(axis=-1)
    targets = np.zeros_like(logits)
    targets[np.arange(B), idx] = 1.0
    aux = (np.logaddexp(0.0, logits) - logits * targets).mean()
    counts = targets.sum(0)
    next_idx = int(np.argmax(counts))
    return np.float32(aux), np.int32(next_idx)
